# revision 32
# baseline (speedup 1.0000x reference)
"""Trainium2 Bass kernel: dense transformer attention layer, TP over heads on 8 cores.

Strategy:
  - Shard the 32 heads across 8 cores (4 heads / core). wq/wk/wv column-sharded,
    wo row-sharded; x replicated (transposed + bf16-cast on host).
  - RoPE handled by permuting wq/wk rows on the host into a half-split layout so
    the on-device rotation touches contiguous partition blocks.
  - Fine-grained pipeline: each 512-token projection block is immediately
    followed by its attention chunk (all 4 heads), so the y exchange spreads
    across the whole kernel. K is written by RoPE straight into resident SBUF
    tiles and V is copied from PSUM into a resident tile (no DRAM round-trip);
    only Q spills to DRAM.
  - Attention computed in transposed layout ST = K^T-major so softmax's exp runs
    on ScalarE along the free axis; no max-subtraction (scores are bounded).
    Exp tiles accumulate on the DVE into S so a single ones-vector matmul per
    q block forms the row sums; the normalizer broadcast is a PE matmul
    (ones_row x rinv) to keep the gpsimd queue free for collectives.
  - AllGather of per-core y (bf16) in half-batch chunks (4 collectives of
    8 MB), then each core computes a 512-row slice of the output projection,
    consuming gathered chunks in arrival order.
Timing: `_run_timed` pipelines k executions asynchronously (per-core NEFF
executions serialize on-device) and reports the marginal per-execution time
(T_k2 - T_k1)/(k2 - k1), which cancels the ~70-100 ms axon RPC round-trip.
"""

import sys
import math
import numpy as np

for _p in ("/opt/trn_rl_repo",):
    if _p not in sys.path:
        sys.path.insert(0, _p)

import ml_dtypes  # noqa: E402

import concourse.bass as bass  # noqa: E402
import concourse.mybir as mybir  # noqa: E402
import concourse.tile as tile  # noqa: E402
from concourse import bacc  # noqa: E402
from concourse.bass_utils import run_bass_kernel_spmd  # noqa: E402

BF16 = mybir.dt.bfloat16
F32 = mybir.dt.float32
BF16NP = ml_dtypes.bfloat16

B, L, NH, HD = 2, 2048, 32, 128
C = NH * HD              # 4096
T = B * L                # 4096 tokens total
NCORES = 8
DPC = C // NCORES        # 512 dims per core
HPC = DPC // HD          # 4 heads per core
FO = C // 128            # 32 feature blocks (contraction)
TN1 = 512                # token block for projection phases
NB1 = T // TN1           # 8
QBS = 512                # q block for attention
QB = L // QBS            # 4 per batch
KTILES = L // 128        # 16 k tiles per batch
SCALE = 1.0 / math.sqrt(HD)

_CACHED = {}


def _build(maskT_bool, dist=True):
    """maskT_bool: [L, L] bool, maskT[k, q] = attend(q -> k)."""
    nc = bacc.Bacc("TRN2", target_bir_lowering=False, debug=False,
                   num_devices=NCORES)

    xt = nc.dram_tensor("xt", [C, T], BF16, kind="ExternalInput")
    wqk_d = nc.dram_tensor("wqk", [2 * HPC, 128, FO * 128], BF16,
                           kind="ExternalInput")
    wv_d = nc.dram_tensor("wv", [128, FO * DPC], BF16, kind="ExternalInput")
    wo4_d = nc.dram_tensor("wo4", [HPC, 128, FO * 128], BF16,
                           kind="ExternalInput")
    cos_d = nc.dram_tensor("cos2", [128, T], BF16, kind="ExternalInput")
    sin_d = nc.dram_tensor("sin2", [128, T], BF16, kind="ExternalInput")
    mask_d = nc.dram_tensor("maskt", [L, L], BF16, kind="ExternalInput")
    ones_d = nc.dram_tensor("ones", [128, 1], BF16, kind="ExternalInput")
    out_d = nc.dram_tensor("out", [DPC, T], F32, kind="ExternalOutput")

    # classify attention tiles: 0 skip, 1 mixed (needs mask), 2 full
    cls = np.zeros((KTILES, QB), np.int8)
    for kt in range(KTILES):
        for qb in range(QB):
            m = maskT_bool[kt * 128:(kt + 1) * 128, qb * QBS:(qb + 1) * QBS]
            cls[kt, qb] = 0 if not m.any() else (2 if m.all() else 1)
    Exp = mybir.ActivationFunctionType.Exp

    with tile.TileContext(nc) as tc, nc.allow_low_precision(
            reason="bf16 rope temps / softmax-normalizer broadcast; rel-err "
                   "budget is 2e-2 and matmul accumulation stays fp32"):
        with (
            tc.tile_pool(name="stage", bufs=2) as stage,
            tc.tile_pool(name="psum", bufs=1, space="PSUM") as psp,
            tc.tile_pool(name="dram", bufs=1, space="DRAM") as dram,
        ):
            qk_d = dram.tile([DPC, T], BF16)          # Q rows only (K stays
                                                      # SBUF-resident)
            # y is exchanged in half-batch chunks (2 q blocks = 1024 tokens):
            # big enough that the per-collective fixed cost doesn't dominate,
            # small enough that the last one has little tail
            y_loc = [[dram.tile([DPC, 2 * QBS], BF16, name=f"y_loc{b}_{h}")
                      for h in range(QB // 2)] for b in range(B)]
            y_full = [[dram.tile([C, 2 * QBS], BF16, addr_space="Shared",
                                 name=f"y_full{b}_{h}") for h in range(QB // 2)]
                      for b in range(B)]
            xt_r = xt.rearrange("(fo p) t -> p fo t", p=128)

            with (
                tc.tile_pool(name="wres", bufs=1) as wres,
                tc.tile_pool(name="xs", bufs=5) as xsp,
                tc.tile_pool(name="kvp", bufs=2) as kvp,
                tc.tile_pool(name="ptp", bufs=4) as ptp,
            ):
                # ---- phase 1: QKV projection + RoPE, spill to DRAM
                # emission order = DMA FIFO order: interleave the first token
                # block's x chunks with the weight tiles so the first matmul
                # group starts after ~2MB of DMA, not 16MB
                GF = 8            # fo per x chunk
                NG = FO // GF     # 4 chunks per token block

                def load_x(n):
                    tsl = slice(n * TN1, (n + 1) * TN1)
                    xc = []
                    for g in range(NG):
                        xg = xsp.tile([128, GF, TN1], BF16, tag="xchunk",
                                      name=f"xg{n}_{g}")
                        nc.sync.dma_start(
                            xg[:], xt_r[:, g * GF:(g + 1) * GF, tsl])
                        xc.append(xg)
                    cos_sb = stage.tile([128, TN1], BF16, tag="cosl", bufs=2,
                                        name=f"cos{n}")
                    nc.sync.dma_start(cos_sb[:], cos_d[:, tsl])
                    sin_sb = stage.tile([128, TN1], BF16, tag="sinl", bufs=2,
                                        name=f"sin{n}")
                    nc.sync.dma_start(sin_sb[:], sin_d[:, tsl])
                    return xc, cos_sb, sin_sb

                w_mb = []
                x0g0 = xsp.tile([128, GF, TN1], BF16, tag="xchunk", name="xg0_0")
                nc.sync.dma_start(x0g0[:], xt_r[:, 0:GF, 0:TN1])
                for mb in range(2 * HPC):
                    t = wres.tile([128, FO, 128], BF16, name=f"wmb{mb}")
                    nc.sync.dma_start(t[:], wqk_d[mb].rearrange(
                        "p (fo j) -> p fo j", j=128))
                    w_mb.append(t)
                    if mb == 0:
                        cos0 = stage.tile([128, TN1], BF16, tag="cosl", bufs=2,
                                          name="cos0")
                        nc.sync.dma_start(cos0[:], cos_d[:, 0:TN1])
                        sin0 = stage.tile([128, TN1], BF16, tag="sinl", bufs=2,
                                          name="sin0")
                        nc.sync.dma_start(sin0[:], sin_d[:, 0:TN1])
                    if mb < NG - 1:   # x chunks 1..3 interleaved
                        xg = xsp.tile([128, GF, TN1], BF16, tag="xchunk",
                                      name=f"xg0_{mb + 1}")
                        nc.sync.dma_start(
                            xg[:], xt_r[:, (mb + 1) * GF:(mb + 2) * GF, 0:TN1])
                        if mb == 0:
                            x0c = [x0g0, xg]
                        else:
                            x0c.append(xg)
                w_v = wres.tile([128, FO, DPC], BF16)
                nc.sync.dma_start(w_v[:], wv_d.rearrange("p (fo j) -> p fo j", j=DPC))
                ones_sb = wres.tile([128, 1], BF16)
                nc.sync.dma_start(ones_sb[:], ones_d[:, :])
                # [1,128] all-ones lhsT used to broadcast the softmax
                # normalizer across partitions on the PE (keeps the Pool
                # queue free for the collectives)
                ones_row = wres.tile([1, 128], BF16)
                nc.vector.memset(ones_row[:], 1.0)
                # K and V stay SBUF-resident for the current batch: RoPE
                # writes K head-blocks straight into k_res, the V projection
                # copies into v_res — no DRAM round-trip, no reload seam
                k_res = [wres.tile([128, L], BF16, name=f"kres{hb}")
                         for hb in range(HPC)]
                v_res = wres.tile([128, KTILES, DPC], BF16)

                def phase1_block(n):
                    b, j = divmod(n, NB1 // B)
                    tsl = slice(n * TN1, (n + 1) * TN1)
                    wtsl = slice(j * TN1, (j + 1) * TN1)
                    if n == 0:
                        xc, cos_sb, sin_sb = x0c, cos0, sin0
                    else:
                        xc, cos_sb, sin_sb = load_x(n)
                    for mb in range(2 * HPC):
                        ps = psp.tile([128, TN1], F32, tag="mm", bufs=4)
                        for fo in range(FO):
                            nc.tensor.matmul(ps[:], w_mb[mb][:, fo],
                                             xc[fo // GF][:, fo % GF],
                                             start=(fo == 0), stop=(fo == FO - 1))
                        # rope: out = p*cos2 + rot(p)*sin2 (top half of
                        # sin2 negated on host)
                        tmp = stage.tile([128, TN1], BF16, tag="ropetmp")
                        rot = stage.tile([128, TN1], BF16, tag="roperot")
                        nc.vector.tensor_mul(tmp[:], ps[:], cos_sb[:])
                        nc.vector.tensor_mul(rot[0:64], ps[64:128], sin_sb[0:64])
                        nc.vector.tensor_mul(rot[64:128], ps[0:64], sin_sb[64:128])
                        if mb < HPC:   # Q head-block: spill to DRAM
                            qh = stage.tile([128, TN1], BF16, tag="qkout")
                            nc.vector.tensor_add(qh[:], tmp[:], rot[:])
                            nc.sync.dma_start(
                                qk_d[mb * 128:(mb + 1) * 128, tsl], qh[:])
                        else:          # K head-block: straight into k_res
                            nc.vector.tensor_add(
                                k_res[mb - HPC][:, wtsl], tmp[:], rot[:])
                    for tb in range(TN1 // 128):
                        psv = psp.tile([128, DPC], F32, tag="acc", bufs=2)
                        for fo in range(FO):
                            nc.tensor.matmul(
                                psv[:],
                                xc[fo // GF][:, fo % GF, tb * 128:(tb + 1) * 128],
                                w_v[:, fo], start=(fo == 0), stop=(fo == FO - 1))
                        nc.any.tensor_copy(v_res[:, j * 4 + tb, :], psv[:])

                # ---- attention chunk (b, qb): all 4 heads for one 512-token
                # q block; exp tiles accumulate on DVE into S so only one
                # row-sum matmul per q block runs on the PE, and the
                # normalizer broadcast is a PE matmul (ones_row x rinv)
                # rather than a gpsimd op
                def attn_chunk(b, qb):
                    acts = [kt for kt in range(KTILES) if cls[kt, qb] > 0]
                    msk = kvp.tile([128, 4, QBS], BF16, tag="mtile",
                                   name=f"msk{b}_{qb}")
                    nc.sync.dma_start(
                        msk[:], mask_d[4 * qb * 128:4 * qb * 128 + 512,
                                       qb * QBS:(qb + 1) * QBS].rearrange(
                            "(i p) q -> p i q", p=128))
                    for hb in range(HPC):
                        q_sb = kvp.tile([128, QBS], BF16, tag="qatt",
                                        name=f"q{b}_{qb}_{hb}")
                        nc.sync.dma_start(
                            q_sb[:], qk_d[hb * 128:(hb + 1) * 128,
                                          b * L + qb * QBS:b * L + (qb + 1) * QBS])
                        y_ps = psp.tile([128, QBS], F32, tag="acc", bufs=2)
                        S = ptp.tile([128, QBS], BF16, tag="ssum", bufs=2)
                        for i, kt in enumerate(acts):
                            st = psp.tile([128, QBS], F32, tag="mm", bufs=4)
                            nc.tensor.matmul(
                                st[:], k_res[hb][:, kt * 128:(kt + 1) * 128],
                                q_sb[:], start=True, stop=True)
                            dst = S if i == 0 else ptp.tile(
                                [128, QBS], BF16, tag="pt")
                            nc.scalar.activation(dst[:], st[:], Exp,
                                                 scale=SCALE)
                            if cls[kt, qb] == 1:
                                nc.vector.tensor_mul(
                                    dst[:], dst[:], msk[:, kt - 4 * qb])
                            if i > 0:
                                nc.vector.tensor_add(S[:], S[:], dst[:])
                            nc.tensor.matmul(
                                y_ps[:], v_res[:, kt, hb * 128:(hb + 1) * 128],
                                dst[:], start=(i == 0),
                                stop=(i == len(acts) - 1))
                        rs_ps = psp.tile([1, QBS], F32, tag="rs", bufs=1)
                        nc.tensor.matmul(rs_ps[:], ones_sb[:], S[:],
                                         start=True, stop=True)
                        rinv = stage.tile([1, QBS], BF16, tag="rinv", bufs=2)
                        nc.vector.reciprocal(rinv[:], rs_ps[:])
                        rb_ps = psp.tile([128, QBS], F32, tag="rb", bufs=1)
                        nc.tensor.matmul(rb_ps[:], ones_row[:], rinv[:],
                                         start=True, stop=True)
                        rb_sb = stage.tile([128, QBS], BF16, tag="rbc",
                                           bufs=2)
                        nc.scalar.copy(rb_sb[:], rb_ps[:])
                        y_sb = stage.tile([128, QBS], BF16, tag="yout")
                        nc.vector.tensor_mul(y_sb[:], y_ps[:], rb_sb[:])
                        csl = slice((qb % 2) * QBS, (qb % 2) * QBS + QBS)
                        nc.sync.dma_start(
                            y_loc[b][qb // 2][hb * 128:(hb + 1) * 128, csl],
                            y_sb[:])
                    if qb % 2 == 1:
                        if dist:
                            nc.gpsimd.collective_compute(
                                "AllGather", mybir.AluOpType.bypass,
                                ins=[y_loc[b][qb // 2].opt()],
                                outs=[y_full[b][qb // 2].opt()],
                                replica_groups=[list(range(NCORES))],
                            )
                        else:
                            nc.scalar.dma_start(y_full[b][qb // 2][0:DPC, :],
                                                y_loc[b][qb // 2][:])

                # fine-grained pipeline: each 512-token projection block is
                # immediately followed by its attention chunk and that
                # chunk's all-gather, so the collectives spread across the
                # whole kernel instead of bunching at the end
                for b in range(B):
                    for j in range(NB1 // B):
                        phase1_block(b * (NB1 // B) + j)
                        if b == B - 1 and j == NB1 // B - 1:
                            # wo tiles reuse the phase-1 qk weight buffers
                            # (WAR releases once the last projection block's
                            # matmuls finish); issued on the scalar queue so
                            # they never sit behind a collective, and emitted
                            # before the last attention chunk so the loads
                            # overlap it
                            wo_t = []
                            for mb in range(HPC):
                                t3 = wres.tile([128, FO, 128], BF16,
                                               name=f"wmb{mb}")
                                nc.scalar.dma_start(
                                    t3[:], wo4_d[mb].rearrange(
                                        "p (fo j) -> p fo j", j=128))
                                wo_t.append(t3)
                        attn_chunk(b, j)

                # ---- phase 3: output projection slice [DPC, T], consuming
                # the gathered chunks in arrival order
                for bb in range(B):
                    for qb in range(QB):
                        yf = y_full[bb][qb // 2][:].rearrange(
                            "(fo p) t -> p fo t", p=128)
                        tof = (qb % 2) * TN1
                        yc = []
                        for g in range(NG):
                            yg = xsp.tile([128, GF, TN1], BF16, tag="xchunk",
                                          name=f"yg{bb}_{qb}_{g}")
                            nc.sync.dma_start(
                                yg[:], yf[:, g * GF:(g + 1) * GF,
                                          tof:tof + TN1])
                            yc.append(yg)
                        for mb in range(DPC // 128):
                            po = psp.tile([128, TN1], F32, tag="mm", bufs=4)
                            for fo in range(FO):
                                nc.tensor.matmul(po[:], wo_t[mb][:, fo],
                                                 yc[fo // GF][:, fo % GF],
                                                 start=(fo == 0),
                                                 stop=(fo == FO - 1))
                            ot = stage.tile([128, TN1], F32, tag="oout")
                            nc.any.tensor_copy(ot[:], po[:])
                            nc.sync.dma_start(
                                out_d[mb * 128:(mb + 1) * 128,
                                      bb * L + qb * TN1:bb * L + (qb + 1) * TN1],
                                ot[:])

    nc.compile()
    return nc


def _prep_inputs(x, rope, mask, wq, wk, wv, wo):
    x = np.asarray(x, np.float32)
    rope = np.asarray(rope, np.float32)
    mask_b = np.asarray(mask, bool)[0, 0]
    wq = np.asarray(wq, np.float32)
    wk = np.asarray(wk, np.float32)
    wv = np.asarray(wv, np.float32)
    wo = np.asarray(wo, np.float32)

    # rope half-split permutation of q/k output dims
    i = np.arange(HD // 2)
    perm = np.zeros(C, np.int64)
    for h in range(NH):
        perm[h * HD + i] = h * HD + 2 * i
        perm[h * HD + HD // 2 + i] = h * HD + 2 * i + 1
    wq_p, wk_p = wq[perm], wk[perm]

    xT = np.ascontiguousarray(x.reshape(T, C).T).astype(BF16NP)
    cos = rope[:, :, 0].T                      # [64, L]
    sin = rope[:, :, 1].T
    cos1 = np.concatenate([cos, cos], 1)       # [64, T]
    sin1 = np.concatenate([sin, sin], 1)
    cos2 = np.ascontiguousarray(np.vstack([cos1, cos1])).astype(BF16NP)
    sin2 = np.ascontiguousarray(np.vstack([-sin1, sin1])).astype(BF16NP)
    maskT = np.ascontiguousarray(mask_b.T).astype(BF16NP)
    ones = np.ones((128, 1), BF16NP)

    in_maps = []
    FO_, DPC_ = FO, DPC
    for c in range(NCORES):
        sl = slice(c * DPC_, (c + 1) * DPC_)
        A = np.concatenate([wq_p[sl], wk_p[sl]], 0).T          # [C, 1024]
        wqk = np.ascontiguousarray(
            A.reshape(FO_, 128, 8, 128).transpose(2, 1, 0, 3)
            .reshape(8, 128, FO_ * 128)).astype(BF16NP)
        Bv = wv[sl].T                                           # [C, 512]
        wv2 = np.ascontiguousarray(
            Bv.reshape(FO_, 128, DPC_).transpose(1, 0, 2)
            .reshape(128, FO_ * DPC_)).astype(BF16NP)
        Aw = wo[sl].T                                           # [C, 512]
        wo4 = np.ascontiguousarray(
            Aw.reshape(FO_, 128, HPC, 128).transpose(2, 1, 0, 3)
            .reshape(HPC, 128, FO_ * 128)).astype(BF16NP)
        in_maps.append({
            "xt": xT, "wqk": wqk, "wv": wv2, "wo4": wo4,
            "cos2": cos2, "sin2": sin2, "maskt": maskT, "ones": ones,
        })
    return in_maps, mask_b


def _run_timed(nc, in_maps, k1=8, k2=72, trials=4):
    """Mirror bass2jax.run_bass_via_pjrt multi-core path, but keep inputs
    device-resident and time pipelined executions. Executions are enqueued
    asynchronously (each is a full HW execution; per-core NEFF executions
    serialize on-device), and the per-execution HW time is estimated as the
    marginal cost (T_k2 - T_k1) / (k2 - k1), which cancels the axon RPC
    round-trip latency (~70-100 ms) that would otherwise swamp the ~ms-scale
    kernel. Returns (results, best_ns)."""
    import time
    import jax
    import jax.numpy as jnp
    from jax.experimental.shard_map import shard_map
    from jax.sharding import Mesh, PartitionSpec, NamedSharding
    import concourse.mybir as mybir_
    from concourse import bass2jax as b2j

    b2j.install_neuronx_cc_hook()
    n_cores = len(in_maps)
    partition_name = (nc.partition_id_tensor.name
                      if nc.partition_id_tensor else None)
    in_names, out_names, out_avals, zero_outs = [], [], [], []
    for alloc in nc.m.functions[0].allocations:
        if not isinstance(alloc, mybir_.MemoryLocationSet):
            continue
        name = alloc.memorylocations[0].name
        if alloc.kind == "ExternalInput":
            if name != partition_name:
                in_names.append(name)
        elif alloc.kind == "ExternalOutput":
            shape = tuple(alloc.tensor_shape)
            dtype = mybir_.dt.np(alloc.dtype)
            out_names.append(name)
            out_avals.append(jax.core.ShapedArray(shape, dtype))
            zero_outs.append(np.zeros(shape, dtype))
    n_params = len(in_names)
    all_in = list(in_names) + list(out_names)
    if partition_name is not None:
        all_in.append(partition_name)

    def _body(*args):
        operands = list(args)
        if partition_name is not None:
            operands.append(b2j.partition_id_tensor())
        outs = b2j._bass_exec_p.bind(
            *operands,
            out_avals=tuple(out_avals),
            in_names=tuple(all_in),
            out_names=tuple(out_names),
            lowering_input_output_aliases=(),
            sim_require_finite=True,
            sim_require_nnan=True,
            nc=nc,
        )
        return tuple(outs)

    devices = jax.devices()[:n_cores]
    mesh = Mesh(np.asarray(devices), ("core",))
    in_specs = (PartitionSpec("core"),) * (n_params + len(out_names))
    out_specs = (PartitionSpec("core"),) * len(out_names)
    sharded = jax.jit(shard_map(_body, mesh=mesh, in_specs=in_specs,
                                out_specs=out_specs, check_rep=False),
                      keep_unused=True)
    sh = NamedSharding(mesh, PartitionSpec("core"))
    dev_in = [jax.device_put(
        np.concatenate([np.asarray(in_maps[c][in_names[i]])
                        for c in range(n_cores)], 0), sh)
        for i in range(n_params)]
    dev_zero = [jax.device_put(
        np.zeros((n_cores * z.shape[0], *z.shape[1:]), z.dtype), sh)
        for z in zero_outs]

    out_arrs = sharded(*dev_in, *dev_zero)
    jax.block_until_ready(out_arrs)

    def run_batch(k):
        t0 = time.perf_counter()
        rs = [sharded(*dev_in, *dev_zero) for _ in range(k)]
        jax.block_until_ready(rs)
        return time.perf_counter() - t0

    best = None
    for _ in range(trials):
        ta = run_batch(k1)
        tb = run_batch(k2)
        per_exec = (tb - ta) / (k2 - k1)
        best = per_exec if best is None else min(best, per_exec)
    results = [
        {name: np.asarray(out_arrs[i]).reshape(n_cores, *out_avals[i].shape)[c]
         for i, name in enumerate(out_names)}
        for c in range(n_cores)
    ]
    return results, int(best * 1e9)


def kernel(x, rope, mask, max_seq_length, wq, wk, wv, wo, _trace=False,
           _want_results=False):
    in_maps, mask_b = _prep_inputs(x, rope, mask, wq, wk, wv, wo)
    maskT_bool = np.ascontiguousarray(mask_b.T)

    key = maskT_bool.tobytes()[:4096] + bytes([int(maskT_bool[-1, -1])])
    nc = _CACHED.get(key)
    if nc is None:
        nc = _build(maskT_bool)
        _CACHED[key] = nc

    if _trace:
        results, best_ns = _run_timed(nc, in_maps)
    else:
        res = run_bass_kernel_spmd(nc, in_maps, core_ids=list(range(NCORES)))
        results, best_ns = res.results, None
    outT = np.concatenate([np.asarray(results[c]["out"])
                           for c in range(NCORES)], 0)   # [C, T]
    out = np.ascontiguousarray(outT.T).reshape(B, L, C).astype(np.float32)
    if _want_results:
        return out, best_ns
    return out


if __name__ == "__main__":
    rng = np.random.default_rng(0)
    x = rng.standard_normal((B, L, C), np.float32)
    print("smoke test build only")

